# revision 8
# baseline (speedup 1.0000x reference)
"""Distributed AlphaFold-style triangle multiplication ("outgoing") on 8
Trainium2 NeuronCores, written in Bass/Tile.

Sharding (per sharding_hint): the 2*C projection channels are sharded across
the 8 cores for the per-channel [768x768]@[768x768]^T einsum; the layernorm +
gated projections before it and the output layernorm/projection/gating after
it run token-sharded (each rank owns 96 t2-rows), linked by two AllToAlls.

V3 notes vs V2:
- A2A#1 is split into 4 token-quarter pieces per channel half (8 collectives),
  each fired from inside P1 as soon as its token range is written, so the
  collective wire time overlaps P1 compute on real HW (the cost model
  serializes collectives; hardware does not).
- mask broadcast moved from gpsimd to sync queue so collective launches on
  gpsimd don't stall the next chunk's mask load.
- P3 ab loads rotate across sync/scalar/gpsimd queues.
- build_nc(reps=N) replicates the body for noise-resistant HW timing.
"""
import sys
sys.path.insert(0, "/opt/trn_rl_repo")
import numpy as np
import ml_dtypes
from contextlib import ExitStack

import concourse.bass as bass
import concourse.tile as tile
from concourse import mybir
from concourse.bass_utils import run_bass_kernel_spmd

NCORES = 8
N = 768
C = 128
TB = N // NCORES            # 96 t2-rows per rank
TOK = N * TB                # 73728 tokens per rank
CH = 512                    # P1 chunk tokens
NCH = TOK // CH             # 144
G = 24                      # stats group (chunks per group)
NGRP = NCH // G             # 6
SOFF = 32                   # s2 rows start partition (PSUM reads need 32-align)
SW = SOFF + G               # stat tile partitions (s1 rows + pad + s2 rows)
QTOK = TOK // 4             # A2A#1 token-quarter (36 chunks of CH)
NG = 4                      # A2A#2 channel groups
CPG = 16 // NG              # 4 channels per group
CH4 = 384                   # P4 chunk tokens (divides 768)
NCH4 = TOK // CH4           # 192
G4 = 24                     # P4 stats group
NGRP4 = NCH4 // G4          # 8
dt = mybir.dt
F32, BF16 = dt.float32, dt.bfloat16
AL = mybir.AluOpType
AF = mybir.ActivationFunctionType


def split_excess_waits(nc, max_waits=1):
    cnt = 0
    for fn in nc.m.functions:
        for bb in fn.blocks:
            insts = list(bb.instructions)
            out = []
            changed = False
            for inst in insts:
                si = inst.sync_info
                if si is not None and si.on_wait and len(si.on_wait) > max_waits:
                    waits = list(si.on_wait)
                    extra, keep = waits[:-max_waits], waits[-max_waits:]
                    for j in range(0, len(extra), max_waits):
                        out.append(mybir.InstNoOp(
                            name=f"{inst.name}_wsplit{j}", ins=[], outs=[],
                            sync_info=mybir.SyncInfo(on_wait=extra[j:j + max_waits], on_update=[]),
                            engine=inst.engine))
                        cnt += 1
                    si.on_wait = keep
                    changed = True
                out.append(inst)
            if changed:
                bb.instructions = out
    return cnt


DEBUG = False


def build_nc(stop_after=99, reps=1):
    nc = bass.Bass("TRN2", target_bir_lowering=False, debug=False, num_devices=NCORES)

    actT = nc.declare_dram_parameter("actT", [C, TOK], BF16, isOutput=False)
    maskT = nc.declare_dram_parameter("maskT", [1, TOK], BF16, isOutput=False)
    # 5 stationary lhsT weights [c, d]: wpa, wpb, wga, wgb, wgl
    wstack = nc.declare_dram_parameter("wstack", [C, 5 * C], BF16, isOutput=False)
    woT = nc.declare_dram_parameter("woT", [C, C], BF16, isOutput=False)
    # small fp32 columns: [cga, cgb, cgl, co]
    cols = nc.declare_dram_parameter("cols", [C, 4], F32, isOutput=False)
    # one-hot stationaries for batched stats: [128, G, 2, SW]
    ohst = nc.declare_dram_parameter("ohst", [C, G * 2 * SW], BF16, isOutput=False)
    # row-selector stationaries for broadcasts: [G, G, 128]
    selst = nc.declare_dram_parameter("selst", [G, G * C], BF16, isOutput=False)
    # row-selector scaled by w_out row sums (rank-1 LN2 bias fold): [G, G, 128]
    cwsel = nc.declare_dram_parameter("cwsel", [G, G * C], BF16, isOutput=False)
    outT = nc.declare_dram_parameter("outT", [C, TOK], BF16, isOutput=True)

    with tile.TileContext(nc) as tc, ExitStack() as ctx:
        dram = ctx.enter_context(tc.tile_pool(name="dram", bufs=1, space="DRAM"))
        wpool = ctx.enter_context(tc.tile_pool(name="wpool", bufs=1))

        # persistent DRAM intermediates
        # p_src rows: (s, c', two) with two=0 -> a-channel, two=1 -> b-channel;
        # half h holds channels c = 8h + c', c' in [0, 8); outer dim = token
        # half th (contiguous per-collective blocks for the split A2A#1)
        p_src_h = [dram.tile([2, 128, TOK // 2], BF16, name=f"p_src{h}") for h in range(2)]
        p_dst_h = [dram.tile([2, 128, TOK // 2], BF16, name=f"p_dst{h}") for h in range(2)]
        tri_src_g = [dram.tile([N, CPG, N], BF16, name=f"tri_src{g}") for g in range(NG)]
        tri_dst = dram.tile([NG, NCORES, TB, CPG, N], BF16, name="tri_dst")
        gT = dram.tile([C, TOK], BF16, name="gT")
        # per-group LN stat rows staged in DRAM for partition-broadcast DMA
        p1_rows = dram.tile([2, NGRP, G, CH], BF16, name="p1_rows")
        p4_rows = dram.tile([NGRP4, G4, CH4], BF16, name="p4_rows")

        # persistent SBUF constants
        wst = wpool.tile([C, 5 * C], BF16)
        nc.sync.dma_start(wst[:], wstack[:, :])
        wo_t = wpool.tile([C, C], BF16)
        nc.sync.dma_start(wo_t[:], woT[:, :])
        colst = wpool.tile([C, 4], F32)
        nc.sync.dma_start(colst[:], cols[:, :])
        cga, cgb, cgl, co = (colst[:, i:i + 1] for i in range(4))
        oh_t = wpool.tile([C, G, 2, SW], BF16)
        nc.sync.dma_start(oh_t[:], ohst[:, :].rearrange("p (g two w) -> p g two w", g=G, two=2))
        cw_t = wpool.tile([G, G, C], BF16)
        nc.sync.dma_start(cw_t[:], cwsel[:, :].rearrange("p (g c) -> p g c", g=G))

        for rep in range(reps):
            emit_body(nc, tc, stop_after,
                      actT, maskT, outT, wst, wo_t, oh_t, cw_t,
                      cga, cgb, cgl, co,
                      p_src_h, p_dst_h, tri_src_g, tri_dst, gT, p1_rows, p4_rows)

    split_excess_waits(nc)
    return nc


def emit_body(nc, tc, stop_after,
              actT, maskT, outT, wst, wo_t, oh_t, cw_t,
              cga, cgb, cgl, co,
              p_src_h, p_dst_h, tri_src_g, tri_dst, gT, p1_rows, p4_rows):
    # ---------------- Phase 1 ----------------
    with tc.tile_pool(name="p1a", bufs=2 * G + 6) as pa_pool, \
         tc.tile_pool(name="p1sb", bufs=6) as sb, \
         tc.tile_pool(name="p1rows", bufs=2) as rows, \
         tc.tile_pool(name="p1ps_s", bufs=1, space="PSUM") as ps_s, \
         tc.tile_pool(name="p1ps_p", bufs=6, space="PSUM") as ps_p:
        for grp in range(NGRP):
            # ---- stage A: load + raw moments (batched into [SW, CH] psum)
            s12 = ps_s.tile([SW, CH], F32, tag="stat")
            a_tiles = []
            for j in range(G):
                ci = grp * G + j
                t0 = ci * CH
                a16 = pa_pool.tile([C, CH], BF16, tag="a16")
                nc.scalar.dma_start(a16[:], actT[:, t0:t0 + CH])
                sq16 = sb.tile([C, CH], BF16, tag="sq16")
                nc.gpsimd.tensor_mul(sq16[:], a16[:], a16[:])
                nc.tensor.matmul(s12[:], oh_t[:, j, 0], a16[:],
                                 start=(j == 0), stop=False)
                nc.tensor.matmul(s12[:], oh_t[:, j, 1], sq16[:],
                                 start=False, stop=(j == G - 1))
                a_tiles.append(a16)
            s1 = s12[0:G, :]
            s2 = s12[SOFF:SOFF + G, :]

            # ---- group row math on [G, CH]
            s1s = rows.tile([G, CH], F32, tag="s1s")
            nc.vector.tensor_copy(s1s[:], s1)
            ss = rows.tile([G, CH], F32, tag="ss")
            nc.vector.tensor_mul(ss[:], s1s[:], s1s[:])
            vare = rows.tile([G, CH], F32, tag="vare")
            nc.vector.scalar_tensor_tensor(
                vare[:], in0=s2, scalar=1e-5, in1=ss[:],
                op0=AL.add, op1=AL.subtract)
            vr = rows.tile([G, CH], F32, tag="vr")
            nc.vector.reciprocal(vr[:], vare[:])
            rstd_bf = rows.tile([G, CH], BF16, tag="rstd_bf")
            nc.scalar.sqrt(rstd_bf[:], vr[:])
            nrsm_bf = rows.tile([G, CH], BF16, tag="nrsm_bf")
            nc.vector.scalar_tensor_tensor(
                nrsm_bf[:], in0=s1, scalar=-1.0, in1=rstd_bf[:],
                op0=AL.mult, op1=AL.mult)
            nc.sync.dma_start(p1_rows[0, grp], rstd_bf[:])
            nc.sync.dma_start(p1_rows[1, grp], nrsm_bf[:])

            # ---- stage C per chunk: broadcast, normalize, project, gate
            for j in range(G):
                ci = grp * G + j
                t0 = ci * CH
                a16 = a_tiles[j]
                bcr_b = sb.tile([C, CH], BF16, tag="bcr_b")
                nc.sync.dma_start(bcr_b[:], p1_rows[0, grp, j:j + 1, :].to_broadcast((C, CH)))
                bcn_b = sb.tile([C, CH], BF16, tag="bcn_b")
                nc.sync.dma_start(bcn_b[:], p1_rows[1, grp, j:j + 1, :].to_broadcast((C, CH)))
                mask_b = sb.tile([C, CH], BF16, tag="mask_b")
                nc.sync.dma_start(mask_b[:], maskT[:, t0:t0 + CH].to_broadcast((C, CH)))

                t16 = sb.tile([C, CH], BF16, tag="t16")
                nc.vector.tensor_mul(t16[:], a16[:], bcr_b[:])
                x16 = sb.tile([C, CH], BF16, tag="x16")
                nc.vector.tensor_tensor(x16[:], t16[:], bcn_b[:], op=AL.add)
                xm16 = sb.tile([C, CH], BF16, tag="xm16")
                nc.vector.tensor_mul(xm16[:], x16[:], mask_b[:])

                pp = {}
                for nm, wi, rhs in [("ga", 2, x16), ("gb", 3, x16), ("gl", 4, x16),
                                    ("pa", 0, xm16), ("pb", 1, xm16)]:
                    ps = ps_p.tile([C, CH], F32, tag="proj")
                    nc.tensor.matmul(ps[:], wst[:, wi * C:(wi + 1) * C], rhs[:],
                                     start=True, stop=True)
                    pp[nm] = ps
                sa16 = sb.tile([C, CH], BF16, tag="sa16")
                nc.scalar.activation(sa16[:], pp["ga"][:], AF.Sigmoid, bias=cga)
                sb16 = sb.tile([C, CH], BF16, tag="sb16")
                nc.scalar.activation(sb16[:], pp["gb"][:], AF.Sigmoid, bias=cgb)
                g16 = sb.tile([C, CH], BF16, tag="g16")
                nc.scalar.activation(g16[:], pp["gl"][:], AF.Sigmoid, bias=cgl)
                pab = sb.tile([C, 2, CH], BF16, tag="pab")
                nc.vector.tensor_mul(pab[:, 0, :], pp["pa"][:], sa16[:])
                nc.vector.tensor_mul(pab[:, 1, :], pp["pb"][:], sb16[:])

                # scatter: pa/pb channels are pre-permuted (host) so half h
                # lives in partitions [64h, 64h+64) as (s, c') = 8s + c';
                # p_src row = 16s + 2c' + two
                th, t0h = ci // (NCH // 2), t0 % (TOK // 2)
                for h in range(2):
                    dsta = p_src_h[h][th].rearrange("(s c two) t -> s c two t",
                                                    s=NCORES, two=2)
                    nc.gpsimd.dma_start(dsta[:, :, :, t0h:t0h + CH],
                                        pab[64 * h:64 * (h + 1)])
                nc.gpsimd.dma_start(gT[:, t0:t0 + CH], g16[:])

                # A2A#1 token-half pieces, fired as soon as their token range
                # is fully written; wire time overlaps remaining P1 compute.
                if (ci + 1) % (NCH // 2) == 0 and stop_after >= 2:
                    for h in range(2):
                        nc.gpsimd.collective_compute(
                            "AllToAll", AL.bypass,
                            replica_groups=[list(range(NCORES))],
                            ins=[p_src_h[h][th].opt()],
                            outs=[p_dst_h[h][th].opt()])

    # ---------------- Phase 3 ----------------
    with tc.tile_pool(name="p3ab", bufs=16) as ab_pool, \
         tc.tile_pool(name="p3out", bufs=4) as sb3o, \
         tc.tile_pool(name="p3ps", bufs=6, space="PSUM") as ps3:
        p3_eng = [nc.sync, nc.scalar, nc.gpsimd]
        # k-packed tiles: contract dim 128 (vs 96) -> 6 accumulation steps
        # instead of 8.  Global k = 96*s + b lives in (sender s, token-half
        # th=b//48); a 128-k tile is a few contiguous (s, th) column runs.
        runs_per_tile = []
        for t in range(6):
            runs = []
            k = 128 * t
            while k < 128 * (t + 1):
                s, b = k // 96, k % 96
                th, bl = b // 48, b % 48
                k1 = min(128 * (t + 1), 96 * s + 48 * (th + 1))
                runs.append((s, th, bl, k - 128 * t, k1 - k))
                k = k1
            runs_per_tile.append(runs)
        ei = 0
        for cc in range(16 if stop_after >= 3 else 0):           # local triangle channel
            g = cc // CPG
            ab_tiles = []
            srcs = [p_dst_h[cc // 8][th].rearrange(
                        "(s c two) (b t) -> s c b two t",
                        s=NCORES, c=8, two=2, b=TB // 2) for th in range(2)]
            for t in range(6):
                ab = ab_pool.tile([C, 2, N], BF16, tag="ab")
                for (s, th, bl, p0, plen) in runs_per_tile[t]:
                    p3_eng[ei % 3].dma_start(
                        ab[p0:p0 + plen], srcs[th][s, cc % 8, bl:bl + plen])
                    ei += 1
                ab_tiles.append(ab)
            for jt in range(6):
                o16 = sb3o.tile([C, N], BF16, tag="o16")
                for i0, iw in ((0, 512), (512, 256)):
                    ps = ps3.tile([C, 512], F32, tag="tri")
                    for t in range(6):
                        nc.tensor.matmul(
                            ps[:, :iw],
                            ab_tiles[t][:, 1, jt * C:(jt + 1) * C],
                            ab_tiles[t][:, 0, i0:i0 + iw],
                            start=(t == 0), stop=(t == 5))
                    if i0 == 0:
                        nc.scalar.activation(o16[:, :iw], ps[:, :iw], AF.Copy)
                    else:
                        nc.vector.tensor_copy(o16[:, i0:i0 + iw], ps[:, :iw])
                nc.sync.dma_start(
                    tri_src_g[g][jt * C:(jt + 1) * C, cc % CPG, :], o16[:])

            # A2A #2 for channel group g as soon as its last channel is out
            if cc % CPG == CPG - 1 and stop_after >= 4:
                nc.gpsimd.collective_compute(
                    "AllToAll", AL.bypass,
                    replica_groups=[list(range(NCORES))],
                    ins=[tri_src_g[g][:].opt()], outs=[tri_dst[g].opt()])

    # ---------------- Phase 4 ----------------
    with tc.tile_pool(name="p4a", bufs=G4 + 4) as p4a, \
         tc.tile_pool(name="p4sb", bufs=4) as sb4, \
         tc.tile_pool(name="p4rows", bufs=2) as rows4, \
         tc.tile_pool(name="p4ps_s", bufs=1, space="PSUM") as ps4s, \
         tc.tile_pool(name="p4ps_o", bufs=5, space="PSUM") as ps4o:
        for grp in range(NGRP4 if stop_after >= 5 else 0):
            s12 = ps4s.tile([SW, CH4], F32, tag="stat")
            tri_tiles = []
            for j in range(G4):
                ci = grp * G4 + j
                jl, i0 = ci // 2, (ci % 2) * CH4
                tri16 = p4a.tile([C, CH4], BF16, tag="tri16")
                nc.sync.dma_start(tri16[:], tri_dst[:, :, jl, :, i0:i0 + CH4])
                sq16 = sb4.tile([C, CH4], BF16, tag="sq16")
                nc.gpsimd.tensor_mul(sq16[:], tri16[:], tri16[:])
                nc.tensor.matmul(s12[:], oh_t[:, j, 0], tri16[:],
                                 start=(j == 0), stop=False)
                nc.tensor.matmul(s12[:], oh_t[:, j, 1, :], sq16[:],
                                 start=False, stop=(j == G4 - 1))
                tri_tiles.append(tri16)
            s1 = s12[0:G4, :]
            s2 = s12[SOFF:SOFF + G4, :]

            s1s = rows4.tile([G4, CH4], F32, tag="s1s")
            nc.vector.tensor_copy(s1s[:], s1)
            negmu_bf = rows4.tile([G4, CH4], BF16, tag="negmu")
            nc.vector.tensor_scalar_mul(negmu_bf[:], s1s[:], -1.0)
            ss = rows4.tile([G4, CH4], F32, tag="ss")
            nc.vector.tensor_mul(ss[:], s1s[:], s1s[:])
            vare = rows4.tile([G4, CH4], F32, tag="vare")
            nc.vector.scalar_tensor_tensor(
                vare[:], in0=s2, scalar=1e-5, in1=ss[:],
                op0=AL.add, op1=AL.subtract)
            vr = rows4.tile([G4, CH4], F32, tag="vr")
            nc.vector.reciprocal(vr[:], vare[:])
            rstd_bf = rows4.tile([G4, CH4], BF16, tag="rstd_bf")
            nc.scalar.sqrt(rstd_bf[:], vr[:])
            nc.scalar.dma_start(p4_rows[grp], rstd_bf[:])

            for j in range(G4):
                ci = grp * G4 + j
                t0 = ci * CH4
                tri16 = tri_tiles[j]
                g16 = sb4.tile([C, CH4], BF16, tag="g16")
                nc.scalar.dma_start(g16[:], gT[:, t0:t0 + CH4])

                bcr_b = sb4.tile([C, CH4], BF16, tag="bcr_b")
                nc.scalar.dma_start(bcr_b[:], p4_rows[grp, j:j + 1, :].to_broadcast((C, CH4)))
                # pso = woT @ tri + w_out_rowsum (x) (-mu)   [rank-1 fold]
                pso = ps4o.tile([C, CH4], F32, tag="o")
                nc.tensor.matmul(pso[:], wo_t[:], tri16[:], start=True, stop=False)
                nc.tensor.matmul(pso[:], cw_t[:, j, :], negmu_bf[:],
                                 start=False, stop=True)

                A = sb4.tile([C, CH4], F32, tag="A")
                nc.vector.tensor_mul(A[:], pso[:], bcr_b[:])
                of16 = sb4.tile([C, CH4], BF16, tag="of16")
                nc.vector.scalar_tensor_tensor(
                    of16[:], in0=A[:], scalar=co, in1=g16[:],
                    op0=AL.add, op1=AL.mult)
                nc.sync.dma_start(outT[:, t0:t0 + CH4], of16[:])


def host_prep(act, mask, ln1_w, ln1_b, w_proj, w_gate, ln2_w, ln2_b, w_out, w_gl):
    bf = ml_dtypes.bfloat16
    act = np.asarray(act, np.float32)
    mask = np.asarray(mask, np.float32)
    w1 = np.asarray(ln1_w, np.float32)
    b1 = np.asarray(ln1_b, np.float32)
    w2 = np.asarray(ln2_w, np.float32)
    b2 = np.asarray(ln2_b, np.float32)
    w_proj = np.asarray(w_proj, np.float32)
    w_gate = np.asarray(w_gate, np.float32)
    w_out = np.asarray(w_out, np.float32)
    w_gl = np.asarray(w_gl, np.float32)
    assert np.all(b1 == 0.0), "nonzero ln1_b not supported in proj path"

    # lhsT weights [c, d] with ln1_w folded
    def lhsT(w):
        return (w.T * w1[:, None]).astype(bf)
    # pa/pb/ga/gb output channels permuted so A2A half h = partitions
    # [64h, 64h+64): new d' = 64*(c//8) + 8*s + c%8 for original d = 16s+c
    perm_p = np.empty(C, np.int64)
    for s in range(NCORES):
        for c in range(16):
            perm_p[64 * (c // 8) + 8 * s + (c % 8)] = 16 * s + c

    def lhsT_p(w):
        return lhsT(w)[:, perm_p]
    wstack = np.concatenate(
        [lhsT_p(w_proj[:C]), lhsT_p(w_proj[C:]), lhsT_p(w_gate[:C]),
         lhsT_p(w_gate[C:]), lhsT(w_gl)],
        axis=1)
    wo_p = w_out * w2[None, :]
    woT = wo_p.T.astype(bf)
    # P4 partition p = 32g + 4s + c'' holds tri channel 16s + 4g + c''
    perm = np.empty(C, np.int64)
    for g in range(4):
        for s in range(8):
            for c2 in range(4):
                perm[32 * g + 4 * s + c2] = 16 * s + 4 * g + c2
    woT = woT[perm]
    wso = wo_p.sum(axis=1)                      # [C] output-channel row sums
    cols = np.stack([
        (w_gate[:C] @ b1)[perm_p], (w_gate[C:] @ b1)[perm_p],
        w_gl @ b1, w_out @ b2], axis=1
    ).astype(np.float32)

    # one-hot stats stationaries [128, G, 2, SW]
    ohst = np.zeros((C, G, 2, SW), np.float32)
    for j in range(G):
        ohst[:, j, 0, j] = 1.0 / C
        ohst[:, j, 1, SOFF + j] = 1.0 / C
    ohst = ohst.reshape(C, G * 2 * SW).astype(bf)
    # row-selector [G, G, 128]: sel[k, j, m] = (k == j)
    selst = np.zeros((G, G, C), np.float32)
    for j in range(G):
        selst[j, j, :] = 1.0
    cwsel = (selst * wso[None, None, :]).reshape(G, G * C).astype(bf)
    selst = selst.reshape(G, G * C).astype(bf)

    in_maps = []
    for r in range(NCORES):
        blk = act[:, TB * r:TB * (r + 1), :]        # [768 t1, 96 t2, 128 c]
        actT = np.ascontiguousarray(blk.transpose(2, 1, 0).reshape(C, TOK)).astype(bf)
        mT = np.ascontiguousarray(mask[:, TB * r:TB * (r + 1)].T.reshape(1, TOK)).astype(bf)
        in_maps.append({"actT": actT, "maskT": mT, "wstack": wstack,
                        "woT": woT, "cols": cols, "ohst": ohst,
                        "selst": selst, "cwsel": cwsel})
    return in_maps


def assemble(results):
    out = np.empty((N, N, C), np.float32)
    for r in range(NCORES):
        o = results[r]["outT"].astype(np.float32).reshape(C, TB, N)
        out[:, TB * r:TB * (r + 1), :] = o.transpose(2, 1, 0)
    return out


_CACHE = {}

def kernel(**inputs):
    if "nc" not in _CACHE:
        _CACHE["nc"] = build_nc()
    in_maps = host_prep(**inputs)
    r = run_bass_kernel_spmd(_CACHE["nc"], in_maps, core_ids=list(range(NCORES)))
    return assemble(r.results)


# revision 11
# speedup vs baseline: 1.4549x; 1.4549x over previous
"""Distributed AlphaFold-style triangle multiplication ("outgoing") on 8
Trainium2 NeuronCores, written in Bass/Tile.

Sharding (per sharding_hint): the 2*C projection channels are sharded across
the 8 cores for the per-channel [768x768]@[768x768]^T einsum; the layernorm +
gated projections before it and the output layernorm/projection/gating after
it run token-sharded (each rank owns 96 t2-rows), linked by two AllToAlls.

V3 notes (on top of the V2 baseline):
- P3 triangle matmul k-tiles are packed to contract dim 128 (vs 96): global
  k = 96*s + b, each 128-k tile is <=2 contiguous sender-column runs of
  p_dst, cutting accumulation steps per output tile from 8 to 6 (-25% PE
  columns) and loading full-128-partition DMA tiles.  Measured -160..-380 us
  vs V2 in interleaved pairwise HW A/B (rel err unchanged at 7.8e-3).
- Token-split A2A#1 fired mid-P1 was tried and REGRESSED ~+400 us on HW
  (collective launches serialize on the gpsimd sequencer); keep the two
  end-of-P1 channel-half collectives.
- build_nc(reps=N) replicates the body for noise-resistant HW timing.
"""
import sys
sys.path.insert(0, "/opt/trn_rl_repo")
import numpy as np
import ml_dtypes
from contextlib import ExitStack

import concourse.bass as bass
import concourse.tile as tile
from concourse import mybir
from concourse.bass_utils import run_bass_kernel_spmd

NCORES = 8
N = 768
C = 128
TB = N // NCORES            # 96 t2-rows per rank
TOK = N * TB                # 73728 tokens per rank
CH = 512                    # P1 chunk tokens
NCH = TOK // CH             # 144
G = 24                      # stats group (chunks per group)
NGRP = NCH // G             # 6
SOFF = 32                   # s2 rows start partition (PSUM reads need 32-align)
SW = SOFF + G               # stat tile partitions (s1 rows + pad + s2 rows)
QTOK = TOK // 4             # A2A#1 token-quarter (36 chunks of CH)
NG = 4                      # A2A#2 channel groups
CPG = 16 // NG              # 4 channels per group
CH4 = 384                   # P4 chunk tokens (divides 768)
NCH4 = TOK // CH4           # 192
G4 = 24                     # P4 stats group
NGRP4 = NCH4 // G4          # 8
dt = mybir.dt
F32, BF16 = dt.float32, dt.bfloat16
AL = mybir.AluOpType
AF = mybir.ActivationFunctionType


def split_excess_waits(nc, max_waits=1):
    cnt = 0
    for fn in nc.m.functions:
        for bb in fn.blocks:
            insts = list(bb.instructions)
            out = []
            changed = False
            for inst in insts:
                si = inst.sync_info
                if si is not None and si.on_wait and len(si.on_wait) > max_waits:
                    waits = list(si.on_wait)
                    extra, keep = waits[:-max_waits], waits[-max_waits:]
                    for j in range(0, len(extra), max_waits):
                        out.append(mybir.InstNoOp(
                            name=f"{inst.name}_wsplit{j}", ins=[], outs=[],
                            sync_info=mybir.SyncInfo(on_wait=extra[j:j + max_waits], on_update=[]),
                            engine=inst.engine))
                        cnt += 1
                    si.on_wait = keep
                    changed = True
                out.append(inst)
            if changed:
                bb.instructions = out
    return cnt


DEBUG = False


def build_nc(stop_after=99, reps=1):
    nc = bass.Bass("TRN2", target_bir_lowering=False, debug=False, num_devices=NCORES)

    actT = nc.declare_dram_parameter("actT", [C, TOK], BF16, isOutput=False)
    maskT = nc.declare_dram_parameter("maskT", [1, TOK], BF16, isOutput=False)
    # 5 stationary lhsT weights [c, d]: wpa, wpb, wga, wgb, wgl
    wstack = nc.declare_dram_parameter("wstack", [C, 5 * C], BF16, isOutput=False)
    woT = nc.declare_dram_parameter("woT", [C, C], BF16, isOutput=False)
    # small fp32 columns: [cga, cgb, cgl, co]
    cols = nc.declare_dram_parameter("cols", [C, 4], F32, isOutput=False)
    # one-hot stationaries for batched stats: [128, G, 2, SW]
    ohst = nc.declare_dram_parameter("ohst", [C, G * 2 * SW], BF16, isOutput=False)
    # row-selector stationaries for broadcasts: [G, G, 128]
    selst = nc.declare_dram_parameter("selst", [G, G * C], BF16, isOutput=False)
    # row-selector scaled by w_out row sums (rank-1 LN2 bias fold): [G, G, 128]
    cwsel = nc.declare_dram_parameter("cwsel", [G, G * C], BF16, isOutput=False)
    outT = nc.declare_dram_parameter("outT", [C, TOK], BF16, isOutput=True)

    with tile.TileContext(nc) as tc, ExitStack() as ctx:
        dram = ctx.enter_context(tc.tile_pool(name="dram", bufs=1, space="DRAM"))
        wpool = ctx.enter_context(tc.tile_pool(name="wpool", bufs=1))

        # persistent DRAM intermediates
        # p_src rows: (s, c', two) with two=0 -> a-channel, two=1 -> b-channel;
        # half h holds channels c = 8h + c', c' in [0, 8); outer dim = token
        # half th (contiguous per-collective blocks for the split A2A#1)
        p_src_h = [dram.tile([128, TOK], BF16, name=f"p_src{h}") for h in range(2)]
        p_dst_h = [dram.tile([128, TOK], BF16, name=f"p_dst{h}") for h in range(2)]
        tri_src_g = [dram.tile([N, CPG, N], BF16, name=f"tri_src{g}") for g in range(NG)]
        tri_dst = dram.tile([NG, NCORES, TB, CPG, N], BF16, name="tri_dst")
        gT = dram.tile([C, TOK], BF16, name="gT")
        # per-group LN stat rows staged in DRAM for partition-broadcast DMA
        p1_rows = dram.tile([2, NGRP, G, CH], BF16, name="p1_rows")
        p4_rows = dram.tile([NGRP4, G4, CH4], BF16, name="p4_rows")

        # persistent SBUF constants
        wst = wpool.tile([C, 5 * C], BF16)
        nc.sync.dma_start(wst[:], wstack[:, :])
        wo_t = wpool.tile([C, C], BF16)
        nc.sync.dma_start(wo_t[:], woT[:, :])
        colst = wpool.tile([C, 4], F32)
        nc.sync.dma_start(colst[:], cols[:, :])
        cga, cgb, cgl, co = (colst[:, i:i + 1] for i in range(4))
        oh_t = wpool.tile([C, G, 2, SW], BF16)
        nc.sync.dma_start(oh_t[:], ohst[:, :].rearrange("p (g two w) -> p g two w", g=G, two=2))
        cw_t = wpool.tile([G, G, C], BF16)
        nc.sync.dma_start(cw_t[:], cwsel[:, :].rearrange("p (g c) -> p g c", g=G))

        for rep in range(reps):
            emit_body(nc, tc, stop_after,
                      actT, maskT, outT, wst, wo_t, oh_t, cw_t,
                      cga, cgb, cgl, co,
                      p_src_h, p_dst_h, tri_src_g, tri_dst, gT, p1_rows, p4_rows)

    split_excess_waits(nc)
    return nc


def emit_body(nc, tc, stop_after,
              actT, maskT, outT, wst, wo_t, oh_t, cw_t,
              cga, cgb, cgl, co,
              p_src_h, p_dst_h, tri_src_g, tri_dst, gT, p1_rows, p4_rows):
    # ---------------- Phase 1 ----------------
    with tc.tile_pool(name="p1a", bufs=2 * G + 6) as pa_pool, \
         tc.tile_pool(name="p1sb", bufs=6) as sb, \
         tc.tile_pool(name="p1rows", bufs=2) as rows, \
         tc.tile_pool(name="p1ps_s", bufs=1, space="PSUM") as ps_s, \
         tc.tile_pool(name="p1ps_p", bufs=6, space="PSUM") as ps_p:
        for grp in range(NGRP):
            # ---- stage A: load + raw moments (batched into [SW, CH] psum)
            s12 = ps_s.tile([SW, CH], F32, tag="stat")
            a_tiles = []
            for j in range(G):
                ci = grp * G + j
                t0 = ci * CH
                a16 = pa_pool.tile([C, CH], BF16, tag="a16")
                nc.scalar.dma_start(a16[:], actT[:, t0:t0 + CH])
                sq16 = sb.tile([C, CH], BF16, tag="sq16")
                nc.vector.tensor_mul(sq16[:], a16[:], a16[:])
                nc.tensor.matmul(s12[:], oh_t[:, j, 0], a16[:],
                                 start=(j == 0), stop=False)
                nc.tensor.matmul(s12[:], oh_t[:, j, 1], sq16[:],
                                 start=False, stop=(j == G - 1))
                a_tiles.append(a16)
            s1 = s12[0:G, :]
            s2 = s12[SOFF:SOFF + G, :]

            # ---- group row math on [G, CH]
            s1s = rows.tile([G, CH], F32, tag="s1s")
            nc.vector.tensor_copy(s1s[:], s1)
            ss = rows.tile([G, CH], F32, tag="ss")
            nc.vector.tensor_mul(ss[:], s1s[:], s1s[:])
            vare = rows.tile([G, CH], F32, tag="vare")
            nc.vector.scalar_tensor_tensor(
                vare[:], in0=s2, scalar=1e-5, in1=ss[:],
                op0=AL.add, op1=AL.subtract)
            vr = rows.tile([G, CH], F32, tag="vr")
            nc.vector.reciprocal(vr[:], vare[:])
            rstd_bf = rows.tile([G, CH], BF16, tag="rstd_bf")
            nc.scalar.sqrt(rstd_bf[:], vr[:])
            nrsm_bf = rows.tile([G, CH], BF16, tag="nrsm_bf")
            nc.vector.scalar_tensor_tensor(
                nrsm_bf[:], in0=s1, scalar=-1.0, in1=rstd_bf[:],
                op0=AL.mult, op1=AL.mult)
            nc.sync.dma_start(p1_rows[0, grp], rstd_bf[:])
            nc.sync.dma_start(p1_rows[1, grp], nrsm_bf[:])

            # ---- stage C per chunk: broadcast, normalize, project, gate
            for j in range(G):
                ci = grp * G + j
                t0 = ci * CH
                a16 = a_tiles[j]
                bcr_b = sb.tile([C, CH], BF16, tag="bcr_b")
                nc.sync.dma_start(bcr_b[:], p1_rows[0, grp, j:j + 1, :].to_broadcast((C, CH)))
                bcn_b = sb.tile([C, CH], BF16, tag="bcn_b")
                nc.sync.dma_start(bcn_b[:], p1_rows[1, grp, j:j + 1, :].to_broadcast((C, CH)))
                mask_b = sb.tile([C, CH], BF16, tag="mask_b")
                nc.gpsimd.dma_start(mask_b[:], maskT[:, t0:t0 + CH].to_broadcast((C, CH)))

                t16 = sb.tile([C, CH], BF16, tag="t16")
                nc.vector.tensor_mul(t16[:], a16[:], bcr_b[:])
                x16 = sb.tile([C, CH], BF16, tag="x16")
                nc.vector.tensor_tensor(x16[:], t16[:], bcn_b[:], op=AL.add)
                xm16 = sb.tile([C, CH], BF16, tag="xm16")
                nc.vector.tensor_mul(xm16[:], x16[:], mask_b[:])

                pp = {}
                for nm, wi, rhs in [("ga", 2, x16), ("gb", 3, x16), ("gl", 4, x16),
                                    ("pa", 0, xm16), ("pb", 1, xm16)]:
                    ps = ps_p.tile([C, CH], F32, tag="proj")
                    nc.tensor.matmul(ps[:], wst[:, wi * C:(wi + 1) * C], rhs[:],
                                     start=True, stop=True)
                    pp[nm] = ps
                sa16 = sb.tile([C, CH], BF16, tag="sa16")
                nc.scalar.activation(sa16[:], pp["ga"][:], AF.Sigmoid, bias=cga)
                sb16 = sb.tile([C, CH], BF16, tag="sb16")
                nc.scalar.activation(sb16[:], pp["gb"][:], AF.Sigmoid, bias=cgb)
                g16 = sb.tile([C, CH], BF16, tag="g16")
                nc.scalar.activation(g16[:], pp["gl"][:], AF.Sigmoid, bias=cgl)
                pab = sb.tile([C, 2, CH], BF16, tag="pab")
                nc.vector.tensor_mul(pab[:, 0, :], pp["pa"][:], sa16[:])
                nc.vector.tensor_mul(pab[:, 1, :], pp["pb"][:], sb16[:])

                # scatter: pa/pb channels are pre-permuted (host) so half h
                # lives in partitions [64h, 64h+64) as (s, c') = 8s + c';
                # p_src row = 16s + 2c' + two
                for h in range(2):
                    dsta = p_src_h[h][:].rearrange("(s c two) t -> s c two t",
                                                   s=NCORES, two=2)
                    nc.gpsimd.dma_start(dsta[:, :, :, t0:t0 + CH],
                                        pab[64 * h:64 * (h + 1)])
                nc.gpsimd.dma_start(gT[:, t0:t0 + CH], g16[:])

                # channel-half A2As once everything is written
                if ci == NCH - 1 and stop_after >= 2:
                    for h in range(2):
                        nc.gpsimd.collective_compute(
                            "AllToAll", AL.bypass,
                            replica_groups=[list(range(NCORES))],
                            ins=[p_src_h[h][:].opt()], outs=[p_dst_h[h][:].opt()])

    # ---------------- Phase 3 ----------------
    with tc.tile_pool(name="p3ab", bufs=16) as ab_pool, \
         tc.tile_pool(name="p3out", bufs=4) as sb3o, \
         tc.tile_pool(name="p3ps", bufs=6, space="PSUM") as ps3:
        # k-packed tiles: contract dim 128 (vs 96) -> 6 accumulation steps
        # instead of 8.  Global k = 96*s + b; each 128-k tile is <=2
        # contiguous sender-column runs of p_dst.
        runs_per_tile = []
        for t in range(6):
            runs, k = [], 128 * t
            while k < 128 * (t + 1):
                s, b = k // 96, k % 96
                k1 = min(128 * (t + 1), 96 * (s + 1))
                runs.append((s, b, k - 128 * t, k1 - k))
                k = k1
            runs_per_tile.append(runs)
        ei = 0
        for cc in range(16 if stop_after >= 3 else 0):           # local triangle channel
            g = cc // CPG
            ab_tiles = []
            src = p_dst_h[cc // 8][:].rearrange(
                "(s c two) (b t) -> s c b two t",
                s=NCORES, c=8, two=2, b=TB)
            for t in range(6):
                ab = ab_pool.tile([C, 2, N], BF16, tag="ab")
                for (s, b, p0, plen) in runs_per_tile[t]:
                    eng = nc.sync if ei % 2 == 0 else nc.scalar
                    eng.dma_start(ab[p0:p0 + plen], src[s, cc % 8, b:b + plen])
                    ei += 1
                ab_tiles.append(ab)
            for jt in range(6):
                o16 = sb3o.tile([C, N], BF16, tag="o16")
                for i0, iw in ((0, 512), (512, 256)):
                    ps = ps3.tile([C, 512], F32, tag="tri")
                    for t in range(6):
                        nc.tensor.matmul(
                            ps[:, :iw],
                            ab_tiles[t][:, 1, jt * C:(jt + 1) * C],
                            ab_tiles[t][:, 0, i0:i0 + iw],
                            start=(t == 0), stop=(t == 5))
                    if i0 == 0:
                        nc.scalar.activation(o16[:, :iw], ps[:, :iw], AF.Copy)
                    else:
                        nc.vector.tensor_copy(o16[:, i0:i0 + iw], ps[:, :iw])
                nc.sync.dma_start(
                    tri_src_g[g][jt * C:(jt + 1) * C, cc % CPG, :], o16[:])

            # A2A #2 for channel group g as soon as its last channel is out
            if cc % CPG == CPG - 1 and stop_after >= 4:
                nc.gpsimd.collective_compute(
                    "AllToAll", AL.bypass,
                    replica_groups=[list(range(NCORES))],
                    ins=[tri_src_g[g][:].opt()], outs=[tri_dst[g].opt()])

    # ---------------- Phase 4 ----------------
    with tc.tile_pool(name="p4a", bufs=G4 + 4) as p4a, \
         tc.tile_pool(name="p4sb", bufs=4) as sb4, \
         tc.tile_pool(name="p4rows", bufs=2) as rows4, \
         tc.tile_pool(name="p4ps_s", bufs=1, space="PSUM") as ps4s, \
         tc.tile_pool(name="p4ps_o", bufs=5, space="PSUM") as ps4o:
        for grp in range(NGRP4 if stop_after >= 5 else 0):
            s12 = ps4s.tile([SW, CH4], F32, tag="stat")
            tri_tiles = []
            for j in range(G4):
                ci = grp * G4 + j
                jl, i0 = ci // 2, (ci % 2) * CH4
                tri16 = p4a.tile([C, CH4], BF16, tag="tri16")
                nc.sync.dma_start(tri16[:], tri_dst[:, :, jl, :, i0:i0 + CH4])
                sq16 = sb4.tile([C, CH4], BF16, tag="sq16")
                nc.gpsimd.tensor_mul(sq16[:], tri16[:], tri16[:])
                nc.tensor.matmul(s12[:], oh_t[:, j, 0], tri16[:],
                                 start=(j == 0), stop=False)
                nc.tensor.matmul(s12[:], oh_t[:, j, 1, :], sq16[:],
                                 start=False, stop=(j == G4 - 1))
                tri_tiles.append(tri16)
            s1 = s12[0:G4, :]
            s2 = s12[SOFF:SOFF + G4, :]

            s1s = rows4.tile([G4, CH4], F32, tag="s1s")
            nc.vector.tensor_copy(s1s[:], s1)
            negmu_bf = rows4.tile([G4, CH4], BF16, tag="negmu")
            nc.vector.tensor_scalar_mul(negmu_bf[:], s1s[:], -1.0)
            ss = rows4.tile([G4, CH4], F32, tag="ss")
            nc.vector.tensor_mul(ss[:], s1s[:], s1s[:])
            vare = rows4.tile([G4, CH4], F32, tag="vare")
            nc.vector.scalar_tensor_tensor(
                vare[:], in0=s2, scalar=1e-5, in1=ss[:],
                op0=AL.add, op1=AL.subtract)
            vr = rows4.tile([G4, CH4], F32, tag="vr")
            nc.vector.reciprocal(vr[:], vare[:])
            rstd_bf = rows4.tile([G4, CH4], BF16, tag="rstd_bf")
            nc.scalar.sqrt(rstd_bf[:], vr[:])
            nc.scalar.dma_start(p4_rows[grp], rstd_bf[:])

            for j in range(G4):
                ci = grp * G4 + j
                t0 = ci * CH4
                tri16 = tri_tiles[j]
                g16 = sb4.tile([C, CH4], BF16, tag="g16")
                nc.scalar.dma_start(g16[:], gT[:, t0:t0 + CH4])

                bcr_b = sb4.tile([C, CH4], BF16, tag="bcr_b")
                nc.scalar.dma_start(bcr_b[:], p4_rows[grp, j:j + 1, :].to_broadcast((C, CH4)))
                # pso = woT @ tri + w_out_rowsum (x) (-mu)   [rank-1 fold]
                pso = ps4o.tile([C, CH4], F32, tag="o")
                nc.tensor.matmul(pso[:], wo_t[:], tri16[:], start=True, stop=False)
                nc.tensor.matmul(pso[:], cw_t[:, j, :], negmu_bf[:],
                                 start=False, stop=True)

                A = sb4.tile([C, CH4], F32, tag="A")
                nc.vector.tensor_mul(A[:], pso[:], bcr_b[:])
                of16 = sb4.tile([C, CH4], BF16, tag="of16")
                nc.vector.scalar_tensor_tensor(
                    of16[:], in0=A[:], scalar=co, in1=g16[:],
                    op0=AL.add, op1=AL.mult)
                nc.sync.dma_start(outT[:, t0:t0 + CH4], of16[:])


def host_prep(act, mask, ln1_w, ln1_b, w_proj, w_gate, ln2_w, ln2_b, w_out, w_gl):
    bf = ml_dtypes.bfloat16
    act = np.asarray(act, np.float32)
    mask = np.asarray(mask, np.float32)
    w1 = np.asarray(ln1_w, np.float32)
    b1 = np.asarray(ln1_b, np.float32)
    w2 = np.asarray(ln2_w, np.float32)
    b2 = np.asarray(ln2_b, np.float32)
    w_proj = np.asarray(w_proj, np.float32)
    w_gate = np.asarray(w_gate, np.float32)
    w_out = np.asarray(w_out, np.float32)
    w_gl = np.asarray(w_gl, np.float32)
    assert np.all(b1 == 0.0), "nonzero ln1_b not supported in proj path"

    # lhsT weights [c, d] with ln1_w folded
    def lhsT(w):
        return (w.T * w1[:, None]).astype(bf)
    # pa/pb/ga/gb output channels permuted so A2A half h = partitions
    # [64h, 64h+64): new d' = 64*(c//8) + 8*s + c%8 for original d = 16s+c
    perm_p = np.empty(C, np.int64)
    for s in range(NCORES):
        for c in range(16):
            perm_p[64 * (c // 8) + 8 * s + (c % 8)] = 16 * s + c

    def lhsT_p(w):
        return lhsT(w)[:, perm_p]
    wstack = np.concatenate(
        [lhsT_p(w_proj[:C]), lhsT_p(w_proj[C:]), lhsT_p(w_gate[:C]),
         lhsT_p(w_gate[C:]), lhsT(w_gl)],
        axis=1)
    wo_p = w_out * w2[None, :]
    woT = wo_p.T.astype(bf)
    # P4 partition p = 32g + 4s + c'' holds tri channel 16s + 4g + c''
    perm = np.empty(C, np.int64)
    for g in range(4):
        for s in range(8):
            for c2 in range(4):
                perm[32 * g + 4 * s + c2] = 16 * s + 4 * g + c2
    woT = woT[perm]
    wso = wo_p.sum(axis=1)                      # [C] output-channel row sums
    cols = np.stack([
        (w_gate[:C] @ b1)[perm_p], (w_gate[C:] @ b1)[perm_p],
        w_gl @ b1, w_out @ b2], axis=1
    ).astype(np.float32)

    # one-hot stats stationaries [128, G, 2, SW]
    ohst = np.zeros((C, G, 2, SW), np.float32)
    for j in range(G):
        ohst[:, j, 0, j] = 1.0 / C
        ohst[:, j, 1, SOFF + j] = 1.0 / C
    ohst = ohst.reshape(C, G * 2 * SW).astype(bf)
    # row-selector [G, G, 128]: sel[k, j, m] = (k == j)
    selst = np.zeros((G, G, C), np.float32)
    for j in range(G):
        selst[j, j, :] = 1.0
    cwsel = (selst * wso[None, None, :]).reshape(G, G * C).astype(bf)
    selst = selst.reshape(G, G * C).astype(bf)

    in_maps = []
    for r in range(NCORES):
        blk = act[:, TB * r:TB * (r + 1), :]        # [768 t1, 96 t2, 128 c]
        actT = np.ascontiguousarray(blk.transpose(2, 1, 0).reshape(C, TOK)).astype(bf)
        mT = np.ascontiguousarray(mask[:, TB * r:TB * (r + 1)].T.reshape(1, TOK)).astype(bf)
        in_maps.append({"actT": actT, "maskT": mT, "wstack": wstack,
                        "woT": woT, "cols": cols, "ohst": ohst,
                        "selst": selst, "cwsel": cwsel})
    return in_maps


def assemble(results):
    out = np.empty((N, N, C), np.float32)
    for r in range(NCORES):
        o = results[r]["outT"].astype(np.float32).reshape(C, TB, N)
        out[:, TB * r:TB * (r + 1), :] = o.transpose(2, 1, 0)
    return out


_CACHE = {}

def kernel(**inputs):
    if "nc" not in _CACHE:
        _CACHE["nc"] = build_nc()
    in_maps = host_prep(**inputs)
    r = run_bass_kernel_spmd(_CACHE["nc"], in_maps, core_ids=list(range(NCORES)))
    return assemble(r.results)
